# revision 2
# baseline (speedup 1.0000x reference)
"""Trainium2 Bass kernel for nn_MessagePassingConvolution (GNN message passing).

Strategy (8 NeuronCores, SPMD):
  * Host: sort edges by receiver (CSR-style), shard the sorted edge stream
    evenly across 8 cores, group each core's edges into node-blocks (<=128
    distinct consecutive node ids per block, padded to a fixed tile count so
    one program serves all cores).
  * Device per core: edge-tile pipeline -
      - MLP gate: feature-on-partition bf16 matmuls (W1/W2), per-edge gate
        via h2-subtile-as-stationary matmul (+ ones x bg bias matmul),
      - equivariant tensor product + gating on VectorE (fused
        scalar_tensor_tensor ops, per-partition attr scalars),
      - scatter-add: one-hot (is_equal vs iota) matmul accumulating into a
        PSUM bank per node-block; flushed PSUM->SBUF->HBM per block.
  * Host: sum per-block 128-row slabs into the [N,512] output (few adds),
    reorder m1 columns to the reference (f-major, c-minor) layout.
  The 1/sqrt(avg_neighbors) normalization and the 1o x 1o -> 0e CG factor are
  folded into Wg/bg, so no extra device work.
"""

import sys

sys.path.insert(0, "/opt/trn_rl_repo")

import numpy as np
from contextlib import ExitStack

from concourse import bacc, tile, bass_utils, mybir

F32 = mybir.dt.float32
BF16 = mybir.dt.bfloat16
AF = mybir.ActivationFunctionType
ALU = mybir.AluOpType

E = 160000
N_NODES = 10000
INV_SQRT3 = 0.5773502691896258
AVG_NUM_NEIGHBORS = 16.0
N_CORES = 8
TILE = 128           # edges per tile (= scatter matmul K)
BK = 12              # tiles per node-block (must be divisible by 4)
ST_TILES = 4         # tiles per supertile (MLP granularity, 512 edges)
BLK_EDGES = BK * TILE

_BF = np.dtype(mybir.dt.np(BF16))


def _to_bf16(x):
    return x.astype(_BF)


# ---------------------------------------------------------------- host prep


def _build_blocks(recv_sorted, lo, hi):
    """Greedy blocking of the sorted edge range [lo, hi): each block covers
    < 128 distinct node ids and at most BLK_EDGES edges. Returns list of
    (base_node, edge_start, edge_end)."""
    blocks = []
    i = lo
    while i < hi:
        base = int(recv_sorted[i])
        limit = np.searchsorted(recv_sorted[lo:hi], base + 128, side="left") + lo
        end = min(i + BLK_EDGES, limit, hi)
        blocks.append((base, i, int(end)))
        i = int(end)
    return blocks


def _build_program(B_max, T_loc):
    """Build the SPMD Bass program: B_max blocks x BK tiles per core."""
    nc = bacc.Bacc("TRN2", target_bir_lowering=False, debug=False,
                   num_devices=N_CORES)
    E_loc = T_loc * TILE

    d_sT = nc.dram_tensor("edge_sT", [64, E_loc], BF16, kind="ExternalInput").ap()
    d_sA = nc.dram_tensor("edge_sA", [128, T_loc * 64], BF16, kind="ExternalInput").ap()
    d_vA = nc.dram_tensor("edge_vA", [128, T_loc * 192], BF16, kind="ExternalInput").ap()
    d_at = nc.dram_tensor("attrs", [128, T_loc * 4], F32, kind="ExternalInput").ap()
    d_rl = nc.dram_tensor("rloc", [128, T_loc], F32, kind="ExternalInput").ap()
    d_io = nc.dram_tensor("iota", [128, 128], BF16, kind="ExternalInput").ap()
    d_w1 = nc.dram_tensor("W1", [64, 128], BF16, kind="ExternalInput").ap()
    d_w2 = nc.dram_tensor("W2", [128, 128], BF16, kind="ExternalInput").ap()
    d_wg = nc.dram_tensor("Wg", [128, 256], BF16, kind="ExternalInput").ap()
    d_b1 = nc.dram_tensor("b1", [128, 1], F32, kind="ExternalInput").ap()
    d_b2 = nc.dram_tensor("b2", [128, 1], F32, kind="ExternalInput").ap()
    d_bg = nc.dram_tensor("bgr", [1, 256], BF16, kind="ExternalInput").ap()
    d_out = nc.dram_tensor("out", [B_max * 128, 512], F32, kind="ExternalOutput").ap()

    with tile.TileContext(nc) as tc, ExitStack() as ctx:
        const = ctx.enter_context(tc.tile_pool(name="const", bufs=1))
        io_pool = ctx.enter_context(tc.tile_pool(name="io", bufs=3))
        mlp_pool = ctx.enter_context(tc.tile_pool(name="mlp", bufs=2))
        gate_pool = ctx.enter_context(tc.tile_pool(name="gate", bufs=4))
        msg_pool = ctx.enter_context(tc.tile_pool(name="msg", bufs=4))
        tmp_pool = ctx.enter_context(tc.tile_pool(name="tmp", bufs=4))
        out_pool = ctx.enter_context(tc.tile_pool(name="outp", bufs=2))
        ps_mlp = ctx.enter_context(tc.tile_pool(name="ps_mlp", bufs=2, space="PSUM"))
        ps_gate = ctx.enter_context(tc.tile_pool(name="ps_gate", bufs=3, space="PSUM"))
        ps_blk = ctx.enter_context(tc.tile_pool(name="ps_blk", bufs=2, space="PSUM"))

        # one-time loads
        t_at = const.tile([128, T_loc * 4], F32, name="t_at")
        t_rl = const.tile([128, T_loc], F32, name="t_rl")
        t_io = const.tile([128, 128], BF16, name="t_io")
        t_w1 = const.tile([64, 128], BF16, name="t_w1")
        t_w2 = const.tile([128, 128], BF16, name="t_w2")
        t_wg = const.tile([128, 256], BF16, name="t_wg")
        t_b1 = const.tile([128, 1], F32, name="t_b1")
        t_b2 = const.tile([128, 1], F32, name="t_b2")
        t_bg = const.tile([1, 256], BF16, name="t_bg")
        t_ones = const.tile([1, 128], BF16, name="t_ones")
        nc.sync.dma_start(t_at[:], d_at[:])
        nc.sync.dma_start(t_rl[:], d_rl[:])
        nc.sync.dma_start(t_io[:], d_io[:])
        nc.sync.dma_start(t_w1[:], d_w1[:])
        nc.sync.dma_start(t_w2[:], d_w2[:])
        nc.sync.dma_start(t_wg[:], d_wg[:])
        nc.sync.dma_start(t_b1[:], d_b1[:])
        nc.sync.dma_start(t_b2[:], d_b2[:])
        nc.sync.dma_start(t_bg[:], d_bg[:])
        nc.vector.memset(t_ones[:], 1.0)

        n_st = (B_max * BK) // ST_TILES
        for b in range(B_max):
            p_blk = ps_blk.tile([128, 512], F32, name=f"p_blk{b}", tag="p_blk")
            for st_in_b in range(BK // ST_TILES):
                st = b * (BK // ST_TILES) + st_in_b
                e0 = st * ST_TILES * TILE  # first edge of supertile

                # ---- loads
                t_sT = io_pool.tile([64, 512], BF16, name=f"sT{st}", tag="sT")
                nc.sync.dma_start(t_sT[:], d_sT[:, e0:e0 + 512])
                t_sA = io_pool.tile([128, 256], BF16, name=f"sA{st}", tag="sA")
                nc.sync.dma_start(t_sA[:], d_sA[:, st * 256:(st + 1) * 256])
                t_vA = io_pool.tile([128, 768], BF16, name=f"vA{st}", tag="vA")
                nc.sync.dma_start(t_vA[:], d_vA[:, st * 768:(st + 1) * 768])

                # ---- MLP (feature-on-partition, bf16)
                p_h1 = ps_mlp.tile([128, 512], F32, name=f"ph1_{st}", tag="p_mlp")
                nc.tensor.matmul(p_h1[:], t_w1[:], t_sT[:], start=True, stop=True)
                t_h1 = mlp_pool.tile([128, 512], BF16, name=f"h1_{st}", tag="h1")
                nc.scalar.activation(t_h1[:], p_h1[:], AF.Silu, bias=t_b1[:, 0:1])
                p_h2 = ps_mlp.tile([128, 512], F32, name=f"ph2_{st}", tag="p_mlp")
                nc.tensor.matmul(p_h2[:], t_w2[:], t_h1[:], start=True, stop=True)
                t_h2 = mlp_pool.tile([128, 512], BF16, name=f"h2_{st}", tag="h2")
                nc.scalar.activation(t_h2[:], p_h2[:], AF.Silu, bias=t_b2[:, 0:1])

                for s in range(ST_TILES):
                    t = st * ST_TILES + s            # global tile index
                    tb = st_in_b * ST_TILES + s      # tile index within block

                    # ---- gate for this 128-edge tile: [128e x 256]
                    p_g = ps_gate.tile([128, 256], F32, name=f"pg{t}", tag="p_g")
                    nc.tensor.matmul(p_g[:], t_h2[:, s * 128:(s + 1) * 128],
                                     t_wg[:], start=True, stop=False)
                    nc.tensor.matmul(p_g[:], t_ones[:], t_bg[:],
                                     start=False, stop=True)
                    t_g = gate_pool.tile([128, 256], BF16, name=f"g{t}", tag="g")
                    nc.vector.tensor_copy(t_g[:], p_g[:])

                    sA = t_sA[:, s * 64:(s + 1) * 64]
                    vA = t_vA[:, s * 192:(s + 1) * 192]
                    a_s = t_at[:, 4 * t + 0:4 * t + 1]
                    av = [t_at[:, 4 * t + 1 + c:4 * t + 2 + c] for c in range(3)]

                    t_msg = msg_pool.tile([128, 512], BF16, name=f"m{t}", tag="m")
                    # m0a = (s * as) * g0a
                    nc.vector.scalar_tensor_tensor(
                        t_msg[:, 0:64], sA, a_s, t_g[:, 0:64], ALU.mult, ALU.mult)
                    # m0b = (sum_c ev_c * av_c) * g0b   (CG factor folded in Wg)
                    t_u = tmp_pool.tile([128, 64], F32, name=f"u{t}", tag="u")
                    t_v = tmp_pool.tile([128, 64], F32, name=f"v{t}", tag="v")
                    t_w = tmp_pool.tile([128, 64], F32, name=f"w{t}", tag="w")
                    nc.vector.tensor_scalar(t_u[:], vA[:, 0:64], av[0], None, ALU.mult)
                    nc.vector.scalar_tensor_tensor(
                        t_v[:], vA[:, 64:128], av[1], t_u[:], ALU.mult, ALU.add)
                    nc.vector.scalar_tensor_tensor(
                        t_w[:], vA[:, 128:192], av[2], t_v[:], ALU.mult, ALU.add)
                    nc.vector.tensor_tensor(
                        t_msg[:, 64:128], t_w[:], t_g[:, 64:128], ALU.mult)
                    # m1a planes: (s * av_c) * g1a
                    for c in range(3):
                        nc.vector.scalar_tensor_tensor(
                            t_msg[:, 128 + 64 * c:192 + 64 * c], sA, av[c],
                            t_g[:, 128:192], ALU.mult, ALU.mult)
                    # m1b = (ev * as) * g1b (repeated per plane)
                    g1b_rep = t_g[:, 192:256].unsqueeze(1).broadcast_to((128, 3, 64))
                    nc.vector.scalar_tensor_tensor(
                        t_msg[:, 320:512].rearrange("p (c v) -> p c v", c=3),
                        vA.rearrange("p (c v) -> p c v", c=3),
                        a_s, g1b_rep, ALU.mult, ALU.mult)

                    # ---- scatter: one-hot matmul accumulate into block PSUM
                    t_oh = tmp_pool.tile([128, 128], BF16, name=f"oh{t}", tag="oh")
                    nc.vector.tensor_scalar(
                        t_oh[:], t_io[:], t_rl[:, t:t + 1], None, ALU.is_equal)
                    nc.tensor.matmul(p_blk[:], t_oh[:], t_msg[:],
                                     start=(tb == 0), stop=(tb == BK - 1))

            # ---- flush block
            t_ob = out_pool.tile([128, 512], F32, name=f"ob{b}", tag="ob")
            nc.scalar.activation(t_ob[:], p_blk[:], AF.Copy)
            nc.sync.dma_start(d_out[b * 128:(b + 1) * 128, :], t_ob[:])

    nc.compile()
    return nc


_PROG_CACHE = {}


def _get_program(B_max, T_loc):
    key = (B_max, T_loc)
    if key not in _PROG_CACHE:
        _PROG_CACHE[key] = _build_program(B_max, T_loc)
    return _PROG_CACHE[key]


def kernel(edge_s, edge_v, attr_s, attr_v, W1, b1, W2, b2, Wg, bg,
           receivers, n_nodes):
    edge_s = np.asarray(edge_s, np.float32)
    edge_v = np.asarray(edge_v, np.float32)
    attr_s = np.asarray(attr_s, np.float32)
    attr_v = np.asarray(attr_v, np.float32)
    W1 = np.asarray(W1, np.float32)
    b1 = np.asarray(b1, np.float32)
    W2 = np.asarray(W2, np.float32)
    b2 = np.asarray(b2, np.float32)
    Wg = np.asarray(Wg, np.float32)
    bg = np.asarray(bg, np.float32)
    receivers = np.asarray(receivers, np.int32)
    n_nodes = int(np.asarray(n_nodes))
    e_total = receivers.shape[0]

    # fold normalization + CG factor into the gate weights
    scale = np.full((256,), 1.0 / np.sqrt(AVG_NUM_NEIGHBORS), np.float32)
    scale[64:128] *= INV_SQRT3
    Wg_f = Wg * scale[None, :]
    bg_f = bg * scale

    # ---- sort by receiver, shard, block
    perm = np.argsort(receivers, kind="stable")
    recv_sorted = receivers[perm]
    cuts = [round(i * e_total / N_CORES) for i in range(N_CORES + 1)]
    core_blocks = [_build_blocks(recv_sorted, cuts[i], cuts[i + 1])
                   for i in range(N_CORES)]
    B_max = max(len(cb) for cb in core_blocks)
    T_loc = B_max * BK
    E_loc = T_loc * TILE

    # ---- per-core packed arrays
    in_maps = []
    meta = []  # per core: list of base nodes
    for ci in range(N_CORES):
        eidx = np.zeros((E_loc,), np.int64)      # gathered edge index (perm'd)
        valid = np.zeros((E_loc,), bool)
        rloc = np.zeros((E_loc,), np.float32)
        bases = []
        for bi, (base, i0, i1) in enumerate(core_blocks[ci]):
            n = i1 - i0
            sl = slice(bi * BLK_EDGES, bi * BLK_EDGES + n)
            eidx[sl] = perm[i0:i1]
            valid[sl] = True
            rloc[sl] = (recv_sorted[i0:i1] - base).astype(np.float32)
            bases.append(base)
        bases += [0] * (B_max - len(bases))
        meta.append(bases)

        es = edge_s[eidx]                       # [E_loc, 64]
        es[~valid] = 0.0
        ev = edge_v[eidx]                       # [E_loc, 64, 3]
        ev[~valid] = 0.0
        a_s = attr_s[eidx, 0]
        a_s[~valid] = 0.0
        a_v = attr_v[eidx]                      # [E_loc, 3]
        a_v[~valid] = 0.0

        ev_pm = np.ascontiguousarray(ev.transpose(0, 2, 1))   # [E_loc, 3, 64]
        attrs4 = np.concatenate([a_s[:, None], a_v], axis=1)  # [E_loc, 4]

        in_maps.append({
            "edge_sT": _to_bf16(np.ascontiguousarray(es.T)),
            "edge_sA": _to_bf16(
                es.reshape(T_loc, TILE, 64).transpose(1, 0, 2).reshape(128, -1)),
            "edge_vA": _to_bf16(
                ev_pm.reshape(T_loc, TILE, 192).transpose(1, 0, 2).reshape(128, -1)),
            "attrs": np.ascontiguousarray(
                attrs4.reshape(T_loc, TILE, 4).transpose(1, 0, 2).reshape(128, -1)),
            "rloc": np.ascontiguousarray(rloc.reshape(T_loc, TILE).T),
            "iota": _to_bf16(np.broadcast_to(
                np.arange(128, dtype=np.float32), (128, 128))),
            "W1": _to_bf16(W1),
            "W2": _to_bf16(W2),
            "Wg": _to_bf16(Wg_f),
            "b1": b1.reshape(128, 1).astype(np.float32),
            "b2": b2.reshape(128, 1).astype(np.float32),
            "bgr": _to_bf16(bg_f.reshape(1, 256)),
        })

    nc = _get_program(B_max, T_loc)
    res = bass_utils.run_bass_kernel_spmd(nc, in_maps, list(range(N_CORES)))

    # ---- host combine: add block slabs, reorder m1 columns
    full = np.zeros((n_nodes + 128, 512), np.float32)
    for ci in range(N_CORES):
        slab = res.results[ci]["out"]
        for bi, base in enumerate(meta[ci]):
            if bi < len(core_blocks[ci]):
                full[base:base + 128] += slab[bi * 128:(bi + 1) * 128]
    full = full[:n_nodes]

    colperm = np.arange(512)
    v = np.arange(64)
    for c in range(3):
        colperm[128 + 3 * v + c] = 128 + 64 * c + v    # m1a
        colperm[320 + 3 * v + c] = 320 + 64 * c + v    # m1b
    return np.ascontiguousarray(full[:, colperm])


# revision 4
# speedup vs baseline: 1.0276x; 1.0276x over previous
"""Trainium2 Bass kernel for nn_MessagePassingConvolution (GNN message passing).

Strategy (8 NeuronCores, SPMD):
  * Host: sort edges by receiver (CSR-style), shard the sorted edge stream
    evenly across 8 cores, group each core's edges into node-blocks (<=128
    distinct consecutive node ids per block, padded to a fixed tile count so
    one program serves all cores).
  * Device per core: edge-tile pipeline -
      - MLP gate: feature-on-partition bf16 matmuls (W1/W2), per-edge gate
        via h2-subtile-as-stationary matmul (+ ones x bg bias matmul),
      - equivariant tensor product + gating on VectorE (fused
        scalar_tensor_tensor ops, per-partition attr scalars),
      - scatter-add: one-hot (is_equal vs iota) matmul accumulating into a
        PSUM bank per node-block; flushed PSUM->SBUF->HBM per block.
  * Host: sum per-block 128-row slabs into the [N,512] output (few adds),
    reorder m1 columns to the reference (f-major, c-minor) layout.
  The 1/sqrt(avg_neighbors) normalization and the 1o x 1o -> 0e CG factor are
  folded into Wg/bg, so no extra device work.
"""

import sys

sys.path.insert(0, "/opt/trn_rl_repo")

import numpy as np
from contextlib import ExitStack

from concourse import bacc, tile, bass_utils, mybir

F32 = mybir.dt.float32
BF16 = mybir.dt.bfloat16
AF = mybir.ActivationFunctionType
ALU = mybir.AluOpType

E = 160000
N_NODES = 10000
INV_SQRT3 = 0.5773502691896258
AVG_NUM_NEIGHBORS = 16.0
N_CORES = 8
TILE = 128           # edges per tile (= scatter matmul K)
BK = 12              # tiles per node-block (must be divisible by 4)
ST_TILES = 4         # tiles per supertile (MLP granularity, 512 edges)
BLK_EDGES = BK * TILE

_BF = np.dtype(mybir.dt.np(BF16))


def _to_bf16(x):
    return x.astype(_BF)


# ---------------------------------------------------------------- host prep


def _build_blocks(recv_sorted, lo, hi):
    """Greedy blocking of the sorted edge range [lo, hi): each block covers
    < 128 distinct node ids and at most BLK_EDGES edges. Returns list of
    (base_node, edge_start, edge_end)."""
    blocks = []
    i = lo
    while i < hi:
        base = int(recv_sorted[i])
        limit = np.searchsorted(recv_sorted[lo:hi], base + 128, side="left") + lo
        end = min(i + BLK_EDGES, limit, hi)
        blocks.append((base, i, int(end)))
        i = int(end)
    return blocks


def _build_program(B_max, T_loc):
    """Build the SPMD Bass program: B_max blocks x BK tiles per core."""
    nc = bacc.Bacc("TRN2", target_bir_lowering=False, debug=False,
                   num_devices=N_CORES)
    E_loc = T_loc * TILE

    d_sT = nc.dram_tensor("edge_sT", [64, E_loc], BF16, kind="ExternalInput").ap()
    d_sA = nc.dram_tensor("edge_sA", [128, T_loc * 64], BF16, kind="ExternalInput").ap()
    d_vA = nc.dram_tensor("edge_vA", [128, T_loc * 192], BF16, kind="ExternalInput").ap()
    d_at = nc.dram_tensor("attrs", [128, T_loc * 4], F32, kind="ExternalInput").ap()
    d_rl = nc.dram_tensor("rloc", [128, T_loc], F32, kind="ExternalInput").ap()
    d_io = nc.dram_tensor("iota", [128, 128], BF16, kind="ExternalInput").ap()
    d_w1 = nc.dram_tensor("W1", [64, 128], BF16, kind="ExternalInput").ap()
    d_w2 = nc.dram_tensor("W2", [128, 128], BF16, kind="ExternalInput").ap()
    d_wg = nc.dram_tensor("Wg", [128, 256], BF16, kind="ExternalInput").ap()
    d_b1 = nc.dram_tensor("b1", [128, 1], F32, kind="ExternalInput").ap()
    d_b2 = nc.dram_tensor("b2", [128, 1], F32, kind="ExternalInput").ap()
    d_bg = nc.dram_tensor("bgr", [1, 256], BF16, kind="ExternalInput").ap()
    d_out = nc.dram_tensor("out", [B_max * 128, 512], F32, kind="ExternalOutput").ap()

    with tile.TileContext(nc) as tc, ExitStack() as ctx:
        const = ctx.enter_context(tc.tile_pool(name="const", bufs=1))
        io_pool = ctx.enter_context(tc.tile_pool(name="io", bufs=3))
        mlp_pool = ctx.enter_context(tc.tile_pool(name="mlp", bufs=2))
        gate_pool = ctx.enter_context(tc.tile_pool(name="gate", bufs=4))
        msg_pool = ctx.enter_context(tc.tile_pool(name="msg", bufs=4))
        tmp_pool = ctx.enter_context(tc.tile_pool(name="tmp", bufs=4))
        out_pool = ctx.enter_context(tc.tile_pool(name="outp", bufs=2))
        ps_mlp = ctx.enter_context(tc.tile_pool(name="ps_mlp", bufs=2, space="PSUM"))
        ps_gate = ctx.enter_context(tc.tile_pool(name="ps_gate", bufs=3, space="PSUM"))
        ps_blk = ctx.enter_context(tc.tile_pool(name="ps_blk", bufs=2, space="PSUM"))

        # one-time loads
        t_at = const.tile([128, T_loc * 4], F32, name="t_at")
        t_rl = const.tile([128, T_loc], F32, name="t_rl")
        t_io = const.tile([128, 128], BF16, name="t_io")
        t_w1 = const.tile([64, 128], BF16, name="t_w1")
        t_w2 = const.tile([128, 128], BF16, name="t_w2")
        t_wg = const.tile([128, 256], BF16, name="t_wg")
        t_b1 = const.tile([128, 1], F32, name="t_b1")
        t_b2 = const.tile([128, 1], F32, name="t_b2")
        t_bg = const.tile([1, 256], BF16, name="t_bg")
        t_ones = const.tile([1, 128], BF16, name="t_ones")
        nc.sync.dma_start(t_at[:], d_at[:])
        nc.sync.dma_start(t_rl[:], d_rl[:])
        nc.sync.dma_start(t_io[:], d_io[:])
        nc.sync.dma_start(t_w1[:], d_w1[:])
        nc.sync.dma_start(t_w2[:], d_w2[:])
        nc.sync.dma_start(t_wg[:], d_wg[:])
        nc.sync.dma_start(t_b1[:], d_b1[:])
        nc.sync.dma_start(t_b2[:], d_b2[:])
        nc.sync.dma_start(t_bg[:], d_bg[:])
        nc.vector.memset(t_ones[:], 1.0)

        n_st = (B_max * BK) // ST_TILES
        for b in range(B_max):
            p_blk = ps_blk.tile([128, 512], F32, name=f"p_blk{b}", tag="p_blk")
            for st_in_b in range(BK // ST_TILES):
                st = b * (BK // ST_TILES) + st_in_b
                e0 = st * ST_TILES * TILE  # first edge of supertile

                # ---- loads
                t_sT = io_pool.tile([64, 512], BF16, name=f"sT{st}", tag="sT")
                nc.sync.dma_start(t_sT[:], d_sT[:, e0:e0 + 512])
                t_sA = io_pool.tile([128, 256], BF16, name=f"sA{st}", tag="sA")
                nc.sync.dma_start(t_sA[:], d_sA[:, st * 256:(st + 1) * 256])
                t_vA = io_pool.tile([128, 768], BF16, name=f"vA{st}", tag="vA")
                nc.sync.dma_start(t_vA[:], d_vA[:, st * 768:(st + 1) * 768])

                # ---- MLP (feature-on-partition, bf16)
                p_h1 = ps_mlp.tile([128, 512], F32, name=f"ph1_{st}", tag="p_mlp")
                nc.tensor.matmul(p_h1[:], t_w1[:], t_sT[:], start=True, stop=True)
                t_h1 = mlp_pool.tile([128, 512], BF16, name=f"h1_{st}", tag="h1")
                nc.scalar.activation(t_h1[:], p_h1[:], AF.Silu, bias=t_b1[:, 0:1])
                p_h2 = ps_mlp.tile([128, 512], F32, name=f"ph2_{st}", tag="p_mlp")
                nc.tensor.matmul(p_h2[:], t_w2[:], t_h1[:], start=True, stop=True)
                t_h2 = mlp_pool.tile([128, 512], BF16, name=f"h2_{st}", tag="h2")
                nc.scalar.activation(t_h2[:], p_h2[:], AF.Silu, bias=t_b2[:, 0:1])

                for s in range(ST_TILES):
                    t = st * ST_TILES + s            # global tile index
                    tb = st_in_b * ST_TILES + s      # tile index within block

                    # ---- gate for this 128-edge tile: [128e x 256]
                    p_g = ps_gate.tile([128, 256], F32, name=f"pg{t}", tag="p_g")
                    nc.tensor.matmul(p_g[:], t_h2[:, s * 128:(s + 1) * 128],
                                     t_wg[:], start=True, stop=False)
                    nc.tensor.matmul(p_g[:], t_ones[:], t_bg[:],
                                     start=False, stop=True)
                    t_g = gate_pool.tile([128, 256], BF16, name=f"g{t}", tag="g")
                    nc.scalar.activation(t_g[:], p_g[:], AF.Copy)

                    sA = t_sA[:, s * 64:(s + 1) * 64]
                    vA = t_vA[:, s * 192:(s + 1) * 192]
                    a_s = t_at[:, 4 * t + 0:4 * t + 1]
                    av = [t_at[:, 4 * t + 1 + c:4 * t + 2 + c] for c in range(3)]

                    t_msg = msg_pool.tile([128, 512], BF16, name=f"m{t}", tag="m")
                    # m0a = (s * as) * g0a
                    nc.vector.scalar_tensor_tensor(
                        t_msg[:, 0:64], sA, a_s, t_g[:, 0:64], ALU.mult, ALU.mult)
                    # m0b = (sum_c ev_c * av_c) * g0b   (CG factor folded in Wg)
                    t_u = tmp_pool.tile([128, 64], BF16, name=f"u{t}", tag="u")
                    t_v = tmp_pool.tile([128, 64], BF16, name=f"v{t}", tag="v")
                    t_w = tmp_pool.tile([128, 64], BF16, name=f"w{t}", tag="w")
                    nc.vector.tensor_scalar(t_u[:], vA[:, 0:64], av[0], None, ALU.mult)
                    nc.vector.scalar_tensor_tensor(
                        t_v[:], vA[:, 64:128], av[1], t_u[:], ALU.mult, ALU.add)
                    nc.vector.scalar_tensor_tensor(
                        t_w[:], vA[:, 128:192], av[2], t_v[:], ALU.mult, ALU.add)
                    nc.vector.tensor_tensor(
                        t_msg[:, 64:128], t_w[:], t_g[:, 64:128], ALU.mult)
                    # m1a planes: (s * av_c) * g1a
                    for c in range(3):
                        nc.vector.scalar_tensor_tensor(
                            t_msg[:, 128 + 64 * c:192 + 64 * c], sA, av[c],
                            t_g[:, 128:192], ALU.mult, ALU.mult)
                    # m1b = (ev * as) * g1b (repeated per plane)
                    g1b_rep = t_g[:, 192:256].unsqueeze(1).broadcast_to((128, 3, 64))
                    nc.vector.scalar_tensor_tensor(
                        t_msg[:, 320:512].rearrange("p (c v) -> p c v", c=3),
                        vA.rearrange("p (c v) -> p c v", c=3),
                        a_s, g1b_rep, ALU.mult, ALU.mult)

                    # ---- scatter: one-hot matmul accumulate into block PSUM
                    t_oh = tmp_pool.tile([128, 128], BF16, name=f"oh{t}", tag="oh")
                    nc.vector.tensor_scalar(
                        t_oh[:], t_io[:], t_rl[:, t:t + 1], None, ALU.is_equal)
                    nc.tensor.matmul(p_blk[:], t_oh[:], t_msg[:],
                                     start=(tb == 0), stop=(tb == BK - 1))

            # ---- flush block
            t_ob = out_pool.tile([128, 512], F32, name=f"ob{b}", tag="ob")
            nc.scalar.activation(t_ob[:], p_blk[:], AF.Copy)
            nc.sync.dma_start(d_out[b * 128:(b + 1) * 128, :], t_ob[:])

    nc.compile()
    return nc


_PROG_CACHE = {}


def _get_program(B_max, T_loc):
    key = (B_max, T_loc)
    if key not in _PROG_CACHE:
        _PROG_CACHE[key] = _build_program(B_max, T_loc)
    return _PROG_CACHE[key]


def kernel(edge_s, edge_v, attr_s, attr_v, W1, b1, W2, b2, Wg, bg,
           receivers, n_nodes):
    edge_s = np.asarray(edge_s, np.float32)
    edge_v = np.asarray(edge_v, np.float32)
    attr_s = np.asarray(attr_s, np.float32)
    attr_v = np.asarray(attr_v, np.float32)
    W1 = np.asarray(W1, np.float32)
    b1 = np.asarray(b1, np.float32)
    W2 = np.asarray(W2, np.float32)
    b2 = np.asarray(b2, np.float32)
    Wg = np.asarray(Wg, np.float32)
    bg = np.asarray(bg, np.float32)
    receivers = np.asarray(receivers, np.int32)
    n_nodes = int(np.asarray(n_nodes))
    e_total = receivers.shape[0]

    # fold normalization + CG factor into the gate weights
    scale = np.full((256,), 1.0 / np.sqrt(AVG_NUM_NEIGHBORS), np.float32)
    scale[64:128] *= INV_SQRT3
    Wg_f = Wg * scale[None, :]
    bg_f = bg * scale

    # ---- sort by receiver, shard, block
    perm = np.argsort(receivers, kind="stable")
    recv_sorted = receivers[perm]
    cuts = [round(i * e_total / N_CORES) for i in range(N_CORES + 1)]
    core_blocks = [_build_blocks(recv_sorted, cuts[i], cuts[i + 1])
                   for i in range(N_CORES)]
    B_max = max(len(cb) for cb in core_blocks)
    T_loc = B_max * BK
    E_loc = T_loc * TILE

    # ---- per-core packed arrays
    in_maps = []
    meta = []  # per core: list of base nodes
    for ci in range(N_CORES):
        eidx = np.zeros((E_loc,), np.int64)      # gathered edge index (perm'd)
        valid = np.zeros((E_loc,), bool)
        rloc = np.zeros((E_loc,), np.float32)
        bases = []
        for bi, (base, i0, i1) in enumerate(core_blocks[ci]):
            n = i1 - i0
            sl = slice(bi * BLK_EDGES, bi * BLK_EDGES + n)
            eidx[sl] = perm[i0:i1]
            valid[sl] = True
            rloc[sl] = (recv_sorted[i0:i1] - base).astype(np.float32)
            bases.append(base)
        bases += [0] * (B_max - len(bases))
        meta.append(bases)

        es = edge_s[eidx]                       # [E_loc, 64]
        es[~valid] = 0.0
        ev = edge_v[eidx]                       # [E_loc, 64, 3]
        ev[~valid] = 0.0
        a_s = attr_s[eidx, 0]
        a_s[~valid] = 0.0
        a_v = attr_v[eidx]                      # [E_loc, 3]
        a_v[~valid] = 0.0

        ev_pm = np.ascontiguousarray(ev.transpose(0, 2, 1))   # [E_loc, 3, 64]
        attrs4 = np.concatenate([a_s[:, None], a_v], axis=1)  # [E_loc, 4]

        in_maps.append({
            "edge_sT": _to_bf16(np.ascontiguousarray(es.T)),
            "edge_sA": _to_bf16(
                es.reshape(T_loc, TILE, 64).transpose(1, 0, 2).reshape(128, -1)),
            "edge_vA": _to_bf16(
                ev_pm.reshape(T_loc, TILE, 192).transpose(1, 0, 2).reshape(128, -1)),
            "attrs": np.ascontiguousarray(
                attrs4.reshape(T_loc, TILE, 4).transpose(1, 0, 2).reshape(128, -1)),
            "rloc": np.ascontiguousarray(rloc.reshape(T_loc, TILE).T),
            "iota": _to_bf16(np.broadcast_to(
                np.arange(128, dtype=np.float32), (128, 128))),
            "W1": _to_bf16(W1),
            "W2": _to_bf16(W2),
            "Wg": _to_bf16(Wg_f),
            "b1": b1.reshape(128, 1).astype(np.float32),
            "b2": b2.reshape(128, 1).astype(np.float32),
            "bgr": _to_bf16(bg_f.reshape(1, 256)),
        })

    nc = _get_program(B_max, T_loc)
    res = bass_utils.run_bass_kernel_spmd(nc, in_maps, list(range(N_CORES)))

    # ---- host combine: add block slabs, reorder m1 columns
    full = np.zeros((n_nodes + 128, 512), np.float32)
    for ci in range(N_CORES):
        slab = res.results[ci]["out"]
        for bi, base in enumerate(meta[ci]):
            if bi < len(core_blocks[ci]):
                full[base:base + 128] += slab[bi * 128:(bi + 1) * 128]
    full = full[:n_nodes]

    colperm = np.arange(512)
    v = np.arange(64)
    for c in range(3):
        colperm[128 + 3 * v + c] = 128 + 64 * c + v    # m1a
        colperm[320 + 3 * v + c] = 320 + 64 * c + v    # m1b
    return np.ascontiguousarray(full[:, colperm])


# revision 13
# speedup vs baseline: 7574.6972x; 7371.1172x over previous
"""Trainium2 Bass kernel for nn_MessagePassingConvolution (GNN message passing).

Strategy (8 NeuronCores, SPMD):
  * Host: sort edges by receiver (CSR-style), shard the sorted edge stream
    evenly across 8 cores, group each core's edges into node-blocks (<=128
    distinct consecutive node ids per block, padded to a fixed tile count so
    one program serves all cores).
  * Device per core: edge-tile pipeline -
      - MLP gate: feature-on-partition bf16 matmuls (W1/W2), per-edge gate
        via h2-subtile-as-stationary matmul (+ ones x bg bias matmul),
      - equivariant tensor product + gating on VectorE (fused
        scalar_tensor_tensor ops, per-partition attr scalars),
      - scatter-add: one-hot (is_equal vs iota) matmul accumulating into a
        PSUM bank per node-block; flushed PSUM->SBUF->HBM per block.
  * Host: sum per-block 128-row slabs into the [N,512] output (few adds),
    reorder m1 columns to the reference (f-major, c-minor) layout.
  The 1/sqrt(avg_neighbors) normalization and the 1o x 1o -> 0e CG factor are
  folded into Wg/bg, so no extra device work.
"""

import sys

sys.path.insert(0, "/opt/trn_rl_repo")

import numpy as np
from contextlib import ExitStack

from concourse import bacc, tile, bass_utils, mybir

F32 = mybir.dt.float32
BF16 = mybir.dt.bfloat16
AF = mybir.ActivationFunctionType
ALU = mybir.AluOpType

E = 160000
N_NODES = 10000
INV_SQRT3 = 0.5773502691896258
AVG_NUM_NEIGHBORS = 16.0
N_CORES = 8
TILE = 128           # edges per tile (= scatter matmul K)
BK = 12              # tiles per node-block (must be divisible by 4)
ST_TILES = 4         # tiles per supertile (MLP granularity, 512 edges)
BLK_EDGES = BK * TILE

_BF = np.dtype(mybir.dt.np(BF16))


def _to_bf16(x):
    return x.astype(_BF)


# ---------------------------------------------------------------- host prep


def _build_blocks(recv_sorted, lo, hi):
    """Greedy blocking of the sorted edge range [lo, hi): each block covers
    < 128 distinct node ids and at most BLK_EDGES edges. Returns list of
    (base_node, edge_start, edge_end)."""
    blocks = []
    i = lo
    while i < hi:
        base = int(recv_sorted[i])
        limit = np.searchsorted(recv_sorted[lo:hi], base + 128, side="left") + lo
        end = min(i + BLK_EDGES, limit, hi)
        blocks.append((base, i, int(end)))
        i = int(end)
    return blocks


OPT = {}


def _build_program(B_max, T_loc, repeat=1):
    """Build the SPMD Bass program: B_max blocks x BK tiles per core.

    repeat > 1 wraps the whole compute in an on-device loop (timing only)."""
    nc = bacc.Bacc("TRN2", target_bir_lowering=False, debug=False,
                   num_devices=N_CORES)
    E_loc = T_loc * TILE

    d_sT = nc.dram_tensor("edge_sT", [64, E_loc], BF16, kind="ExternalInput").ap()
    d_sA = nc.dram_tensor("edge_sA", [128, T_loc * 64], BF16, kind="ExternalInput").ap()
    d_vA = nc.dram_tensor("edge_vA", [128, T_loc * 192], BF16, kind="ExternalInput").ap()
    d_at = nc.dram_tensor("attrs", [128, T_loc * 4], F32, kind="ExternalInput").ap()
    d_rl = nc.dram_tensor("rloc", [128, T_loc], F32, kind="ExternalInput").ap()
    d_io = nc.dram_tensor("iota", [128, 128], BF16, kind="ExternalInput").ap()
    d_w1 = nc.dram_tensor("W1", [64, 128], BF16, kind="ExternalInput").ap()
    d_w2 = nc.dram_tensor("W2", [128, 128], BF16, kind="ExternalInput").ap()
    d_wg = nc.dram_tensor("Wg", [128, 256], BF16, kind="ExternalInput").ap()
    d_b1 = nc.dram_tensor("b1", [128, 1], F32, kind="ExternalInput").ap()
    d_b2 = nc.dram_tensor("b2", [128, 1], F32, kind="ExternalInput").ap()
    d_bg = nc.dram_tensor("bgr", [1, 256], BF16, kind="ExternalInput").ap()
    d_out = nc.dram_tensor("out", [B_max * 128, 512], F32, kind="ExternalOutput").ap()

    with tile.TileContext(nc) as tc, ExitStack() as ctx:
        const = ctx.enter_context(tc.tile_pool(name="const", bufs=1))
        io_pool = ctx.enter_context(tc.tile_pool(name="io", bufs=3))
        mlp_pool = ctx.enter_context(tc.tile_pool(name="mlp", bufs=2))
        gate_pool = ctx.enter_context(tc.tile_pool(name="gate", bufs=4))
        msg_pool = ctx.enter_context(tc.tile_pool(name="msg", bufs=4))
        tmp_pool = ctx.enter_context(tc.tile_pool(name="tmp", bufs=4))
        out_pool = ctx.enter_context(tc.tile_pool(name="outp", bufs=2))
        ps_mlp = ctx.enter_context(tc.tile_pool(name="ps_mlp", bufs=2, space="PSUM"))
        ps_gate = ctx.enter_context(tc.tile_pool(name="ps_gate", bufs=3, space="PSUM"))
        ps_blk = ctx.enter_context(tc.tile_pool(name="ps_blk", bufs=2, space="PSUM"))

        # one-time loads
        t_at = const.tile([128, T_loc * 4], F32, name="t_at")
        t_rl = const.tile([128, T_loc], F32, name="t_rl")
        t_io = const.tile([128, 128], BF16, name="t_io")
        t_w1 = const.tile([64, 128], BF16, name="t_w1")
        t_w2 = const.tile([128, 128], BF16, name="t_w2")
        t_wg = const.tile([128, 256], BF16, name="t_wg")
        t_b1 = const.tile([128, 1], F32, name="t_b1")
        t_b2 = const.tile([128, 1], F32, name="t_b2")
        t_bg = const.tile([1, 256], BF16, name="t_bg")
        t_ones = const.tile([1, 128], BF16, name="t_ones")
        nc.sync.dma_start(t_at[:], d_at[:])
        nc.sync.dma_start(t_rl[:], d_rl[:])
        nc.sync.dma_start(t_io[:], d_io[:])
        nc.sync.dma_start(t_w1[:], d_w1[:])
        nc.sync.dma_start(t_w2[:], d_w2[:])
        nc.sync.dma_start(t_wg[:], d_wg[:])
        nc.sync.dma_start(t_b1[:], d_b1[:])
        nc.sync.dma_start(t_b2[:], d_b2[:])
        nc.sync.dma_start(t_bg[:], d_bg[:])
        nc.vector.memset(t_ones[:], 1.0)

        loop_ctx = tc.For_i(0, repeat, 1) if repeat > 1 else None
        if loop_ctx is not None:
            ctx.enter_context(loop_ctx)
        for b in range(B_max):
            p_blk = ps_blk.tile([128, 512], F32, name=f"p_blk{b}", tag="p_blk")
            for st_in_b in range(BK // ST_TILES):
                st = b * (BK // ST_TILES) + st_in_b
                e0 = st * ST_TILES * TILE  # first edge of supertile

                # ---- loads
                t_sT = io_pool.tile([64, 512], BF16, name=f"sT{st}", tag="sT")
                nc.sync.dma_start(t_sT[:], d_sT[:, e0:e0 + 512])
                t_sA = io_pool.tile([128, 256], BF16, name=f"sA{st}", tag="sA")
                nc.sync.dma_start(t_sA[:], d_sA[:, st * 256:(st + 1) * 256])
                t_vA = io_pool.tile([128, 768], BF16, name=f"vA{st}", tag="vA")
                nc.sync.dma_start(t_vA[:], d_vA[:, st * 768:(st + 1) * 768])

                # ---- MLP (feature-on-partition, bf16)
                p_h1 = ps_mlp.tile([128, 512], F32, name=f"ph1_{st}", tag="p_mlp")
                nc.tensor.matmul(p_h1[:], t_w1[:], t_sT[:], start=True, stop=True)
                t_h1 = mlp_pool.tile([128, 512], BF16, name=f"h1_{st}", tag="h1")
                nc.scalar.activation(t_h1[:], p_h1[:], AF.Silu, bias=t_b1[:, 0:1])
                p_h2 = ps_mlp.tile([128, 512], F32, name=f"ph2_{st}", tag="p_mlp")
                nc.tensor.matmul(p_h2[:], t_w2[:], t_h1[:], start=True, stop=True)
                t_h2 = mlp_pool.tile([128, 512], BF16, name=f"h2_{st}", tag="h2")
                nc.scalar.activation(t_h2[:], p_h2[:], AF.Silu, bias=t_b2[:, 0:1])

                # gate pairs: two subtiles share one [128x512] PSUM bank so
                # the ACT copy runs once per pair
                use_bias = OPT.get("gate_bias", True)
                gpair = []
                for half in range(2):
                    p_g2 = ps_gate.tile([128, 512], F32, name=f"pg{st}_{half}",
                                        tag="p_g")
                    for q in range(2):
                        s = half * 2 + q
                        nc.tensor.matmul(
                            p_g2[:, q * 256:(q + 1) * 256],
                            t_h2[:, s * 128:(s + 1) * 128], t_wg[:],
                            start=True, stop=not use_bias)
                        if use_bias:
                            nc.tensor.matmul(
                                p_g2[:, q * 256:(q + 1) * 256], t_ones[:],
                                t_bg[:], start=False, stop=True)
                    t_g2 = gate_pool.tile([128, 512], BF16, name=f"g{st}_{half}",
                                          tag="g")
                    nc.scalar.activation(t_g2[:], p_g2[:], AF.Copy)
                    gpair.append(t_g2)

                for s in range(ST_TILES):
                    t = st * ST_TILES + s            # global tile index
                    tb = st_in_b * ST_TILES + s      # tile index within block
                    t_g = gpair[s // 2][:, (s % 2) * 256:(s % 2 + 1) * 256]

                    sA = t_sA[:, s * 64:(s + 1) * 64]
                    vA = t_vA[:, s * 192:(s + 1) * 192]
                    a_s = t_at[:, 4 * t + 0:4 * t + 1]
                    av = [t_at[:, 4 * t + 1 + c:4 * t + 2 + c] for c in range(3)]

                    t_msg = msg_pool.tile([128, 512], BF16, name=f"m{t}", tag="m")
                    # m0a = (s * as) * g0a
                    nc.vector.scalar_tensor_tensor(
                        t_msg[:, 0:64], sA, a_s, t_g[:, 0:64], ALU.mult, ALU.mult)
                    # m0b = (sum_c ev_c * av_c) * g0b   (CG factor folded in Wg)
                    t_u = tmp_pool.tile([128, 64], BF16, name=f"u{t}", tag="u")
                    t_v = tmp_pool.tile([128, 64], BF16, name=f"v{t}", tag="v")
                    t_w = tmp_pool.tile([128, 64], BF16, name=f"w{t}", tag="w")
                    nc.vector.tensor_scalar(t_u[:], vA[:, 0:64], av[0], None, ALU.mult)
                    nc.vector.scalar_tensor_tensor(
                        t_v[:], vA[:, 64:128], av[1], t_u[:], ALU.mult, ALU.add)
                    nc.vector.scalar_tensor_tensor(
                        t_w[:], vA[:, 128:192], av[2], t_v[:], ALU.mult, ALU.add)
                    nc.vector.tensor_tensor(
                        t_msg[:, 64:128], t_w[:], t_g[:, 64:128], ALU.mult)
                    # m1a planes: (s * av_c) * g1a
                    m1a_eng = nc.gpsimd if OPT.get("m1a_gpsimd") else nc.vector
                    for c in range(3):
                        m1a_eng.scalar_tensor_tensor(
                            t_msg[:, 128 + 64 * c:192 + 64 * c], sA, av[c],
                            t_g[:, 128:192], ALU.mult, ALU.mult)
                    # m1b = (ev * as) * g1b (repeated per plane)
                    m1b_eng = nc.gpsimd if OPT.get("m1b_gpsimd") else nc.vector
                    g1b_rep = t_g[:, 192:256].unsqueeze(1).broadcast_to((128, 3, 64))
                    m1b_eng.scalar_tensor_tensor(
                        t_msg[:, 320:512].rearrange("p (c v) -> p c v", c=3),
                        vA.rearrange("p (c v) -> p c v", c=3),
                        a_s, g1b_rep, ALU.mult, ALU.mult)

                    # ---- scatter: one-hot matmul accumulate into block PSUM
                    t_oh = tmp_pool.tile([128, 128], BF16, name=f"oh{t}", tag="oh")
                    nc.vector.tensor_scalar(
                        t_oh[:], t_io[:], t_rl[:, t:t + 1], None, ALU.is_equal)
                    nc.tensor.matmul(p_blk[:], t_oh[:], t_msg[:],
                                     start=(tb == 0), stop=(tb == BK - 1))

            # ---- flush block
            t_ob = out_pool.tile([128, 512], F32, name=f"ob{b}", tag="ob")
            nc.scalar.activation(t_ob[:], p_blk[:], AF.Copy)
            nc.sync.dma_start(d_out[b * 128:(b + 1) * 128, :], t_ob[:])

    nc.compile()
    return nc


_PROG_CACHE = {}


def _get_program(B_max, T_loc, gate_bias):
    key = (B_max, T_loc, gate_bias)
    if key not in _PROG_CACHE:
        OPT["gate_bias"] = gate_bias
        _PROG_CACHE[key] = _build_program(B_max, T_loc)
    return _PROG_CACHE[key]


def kernel(edge_s, edge_v, attr_s, attr_v, W1, b1, W2, b2, Wg, bg,
           receivers, n_nodes):
    edge_s = np.asarray(edge_s, np.float32)
    edge_v = np.asarray(edge_v, np.float32)
    attr_s = np.asarray(attr_s, np.float32)
    attr_v = np.asarray(attr_v, np.float32)
    W1 = np.asarray(W1, np.float32)
    b1 = np.asarray(b1, np.float32)
    W2 = np.asarray(W2, np.float32)
    b2 = np.asarray(b2, np.float32)
    Wg = np.asarray(Wg, np.float32)
    bg = np.asarray(bg, np.float32)
    receivers = np.asarray(receivers, np.int32)
    n_nodes = int(np.asarray(n_nodes))
    e_total = receivers.shape[0]

    # fold normalization + CG factor into the gate weights
    scale = np.full((256,), 1.0 / np.sqrt(AVG_NUM_NEIGHBORS), np.float32)
    scale[64:128] *= INV_SQRT3
    Wg_f = Wg * scale[None, :]
    bg_f = bg * scale

    # ---- sort by receiver, shard, block
    perm = np.argsort(receivers, kind="stable")
    recv_sorted = receivers[perm]
    cuts = [round(i * e_total / N_CORES) for i in range(N_CORES + 1)]
    core_blocks = [_build_blocks(recv_sorted, cuts[i], cuts[i + 1])
                   for i in range(N_CORES)]
    B_max = max(len(cb) for cb in core_blocks)
    T_loc = B_max * BK
    E_loc = T_loc * TILE

    # ---- per-core packed arrays
    in_maps = []
    meta = []  # per core: list of base nodes
    for ci in range(N_CORES):
        eidx = np.zeros((E_loc,), np.int64)      # gathered edge index (perm'd)
        valid = np.zeros((E_loc,), bool)
        rloc = np.zeros((E_loc,), np.float32)
        bases = []
        for bi, (base, i0, i1) in enumerate(core_blocks[ci]):
            n = i1 - i0
            sl = slice(bi * BLK_EDGES, bi * BLK_EDGES + n)
            eidx[sl] = perm[i0:i1]
            valid[sl] = True
            rloc[sl] = (recv_sorted[i0:i1] - base).astype(np.float32)
            bases.append(base)
        bases += [0] * (B_max - len(bases))
        meta.append(bases)

        es = edge_s[eidx]                       # [E_loc, 64]
        es[~valid] = 0.0
        ev = edge_v[eidx]                       # [E_loc, 64, 3]
        ev[~valid] = 0.0
        a_s = attr_s[eidx, 0]
        a_s[~valid] = 0.0
        a_v = attr_v[eidx]                      # [E_loc, 3]
        a_v[~valid] = 0.0

        ev_pm = np.ascontiguousarray(ev.transpose(0, 2, 1))   # [E_loc, 3, 64]
        attrs4 = np.concatenate([a_s[:, None], a_v], axis=1)  # [E_loc, 4]

        in_maps.append({
            "edge_sT": _to_bf16(np.ascontiguousarray(es.T)),
            "edge_sA": _to_bf16(
                es.reshape(T_loc, TILE, 64).transpose(1, 0, 2).reshape(128, -1)),
            "edge_vA": _to_bf16(
                ev_pm.reshape(T_loc, TILE, 192).transpose(1, 0, 2).reshape(128, -1)),
            "attrs": np.ascontiguousarray(
                attrs4.reshape(T_loc, TILE, 4).transpose(1, 0, 2).reshape(128, -1)),
            "rloc": np.ascontiguousarray(rloc.reshape(T_loc, TILE).T),
            "iota": _to_bf16(np.broadcast_to(
                np.arange(128, dtype=np.float32), (128, 128))),
            "W1": _to_bf16(W1),
            "W2": _to_bf16(W2),
            "Wg": _to_bf16(Wg_f),
            "b1": b1.reshape(128, 1).astype(np.float32),
            "b2": b2.reshape(128, 1).astype(np.float32),
            "bgr": _to_bf16(bg_f.reshape(1, 256)),
        })

    nc = _get_program(B_max, T_loc, gate_bias=bool(np.any(bg_f != 0)))
    res = bass_utils.run_bass_kernel_spmd(nc, in_maps, list(range(N_CORES)))

    # ---- host combine: add block slabs, reorder m1 columns
    full = np.zeros((n_nodes + 128, 512), np.float32)
    for ci in range(N_CORES):
        slab = res.results[ci]["out"]
        for bi, base in enumerate(meta[ci]):
            if bi < len(core_blocks[ci]):
                full[base:base + 128] += slab[bi * 128:(bi + 1) * 128]
    full = full[:n_nodes]

    colperm = np.arange(512)
    v = np.arange(64)
    for c in range(3):
        colperm[128 + 3 * v + c] = 128 + 64 * c + v    # m1a
        colperm[320 + 3 * v + c] = 320 + 64 * c + v    # m1b
    return np.ascontiguousarray(full[:, colperm])


# revision 17
# speedup vs baseline: 8572.3620x; 1.1317x over previous
"""Trainium2 Bass kernel for nn_MessagePassingConvolution (GNN message passing).

Strategy (8 NeuronCores, SPMD):
  * Host: sort edges by receiver (CSR-style), shard the sorted edge stream
    evenly across 8 cores, group each core's edges into node-blocks (<=128
    distinct consecutive node ids per block, padded to a fixed tile count so
    one program serves all cores).
  * Device per core: edge-tile pipeline -
      - MLP gate: feature-on-partition bf16 matmuls (W1/W2), per-edge gate
        via h2-subtile-as-stationary matmul (+ ones x bg bias matmul),
      - equivariant tensor product + gating on VectorE (fused
        scalar_tensor_tensor ops, per-partition attr scalars),
      - scatter-add: one-hot (is_equal vs iota) matmul accumulating into a
        PSUM bank per node-block; flushed PSUM->SBUF->HBM per block.
  * Host: sum per-block 128-row slabs into the [N,512] output (few adds),
    reorder m1 columns to the reference (f-major, c-minor) layout.
  The 1/sqrt(avg_neighbors) normalization and the 1o x 1o -> 0e CG factor are
  folded into Wg/bg, so no extra device work.
"""

import sys

sys.path.insert(0, "/opt/trn_rl_repo")

import numpy as np
from contextlib import ExitStack

from concourse import bacc, tile, bass_utils, mybir

F32 = mybir.dt.float32
BF16 = mybir.dt.bfloat16
AF = mybir.ActivationFunctionType
ALU = mybir.AluOpType

E = 160000
N_NODES = 10000
INV_SQRT3 = 0.5773502691896258
AVG_NUM_NEIGHBORS = 16.0
N_CORES = 8
TILE = 128           # edges per tile (= scatter matmul K)
BK = 8               # tiles per node-block (must be divisible by ST_TILES)
ST_TILES = 8         # tiles per supertile (MLP granularity, 1024 edges)
BLK_EDGES = BK * TILE
ST_E = ST_TILES * TILE

_BF = np.dtype(mybir.dt.np(BF16))


def _to_bf16(x):
    return x.astype(_BF)


# ---------------------------------------------------------------- host prep


def _build_blocks(recv_sorted, lo, hi):
    """Greedy blocking of the sorted edge range [lo, hi): each block covers
    < 128 distinct node ids and at most BLK_EDGES edges. Returns list of
    (base_node, edge_start, edge_end)."""
    blocks = []
    i = lo
    while i < hi:
        base = int(recv_sorted[i])
        limit = np.searchsorted(recv_sorted[lo:hi], base + 128, side="left") + lo
        end = min(i + BLK_EDGES, limit, hi)
        blocks.append((base, i, int(end)))
        i = int(end)
    return blocks


OPT = {}


def _build_program(B_max, T_loc, repeat=1):
    """Build the SPMD Bass program: B_max blocks x BK tiles per core.

    repeat > 1 wraps the whole compute in an on-device loop (timing only)."""
    nc = bacc.Bacc("TRN2", target_bir_lowering=False, debug=False,
                   num_devices=N_CORES)
    E_loc = T_loc * TILE

    d_sT = nc.dram_tensor("edge_sT", [64, E_loc], BF16, kind="ExternalInput").ap()
    d_sA = nc.dram_tensor("edge_sA", [128, T_loc * 64], BF16, kind="ExternalInput").ap()
    d_vA = nc.dram_tensor("edge_vA", [128, T_loc * 192], BF16, kind="ExternalInput").ap()
    d_at = nc.dram_tensor("attrs", [128, T_loc * 4], F32, kind="ExternalInput").ap()
    d_rl = nc.dram_tensor("rloc", [128, T_loc], F32, kind="ExternalInput").ap()
    d_io = nc.dram_tensor("iota", [128, 128], BF16, kind="ExternalInput").ap()
    d_w1 = nc.dram_tensor("W1", [64, 128], BF16, kind="ExternalInput").ap()
    d_w2 = nc.dram_tensor("W2", [128, 128], BF16, kind="ExternalInput").ap()
    d_wg = nc.dram_tensor("Wg", [128, 256], BF16, kind="ExternalInput").ap()
    d_b1 = nc.dram_tensor("b1", [128, 1], F32, kind="ExternalInput").ap()
    d_b2 = nc.dram_tensor("b2", [128, 1], F32, kind="ExternalInput").ap()
    d_bg = nc.dram_tensor("bgr", [1, 256], BF16, kind="ExternalInput").ap()
    d_out = nc.dram_tensor("out", [B_max * 128, 512], F32, kind="ExternalOutput").ap()

    with tile.TileContext(nc) as tc, ExitStack() as ctx:
        const = ctx.enter_context(tc.tile_pool(name="const", bufs=1))
        io_pool = ctx.enter_context(tc.tile_pool(name="io", bufs=3))
        mlp_pool = ctx.enter_context(tc.tile_pool(name="mlp", bufs=2))
        gate_pool = ctx.enter_context(tc.tile_pool(name="gate", bufs=4))
        msg_pool = ctx.enter_context(tc.tile_pool(name="msg", bufs=4))
        tmp_pool = ctx.enter_context(tc.tile_pool(name="tmp", bufs=4))
        out_pool = ctx.enter_context(tc.tile_pool(name="outp", bufs=2))
        ps_mlp = ctx.enter_context(tc.tile_pool(name="ps_mlp", bufs=2, space="PSUM"))
        ps_gate = ctx.enter_context(tc.tile_pool(name="ps_gate", bufs=2, space="PSUM"))
        ps_blk = ctx.enter_context(tc.tile_pool(name="ps_blk", bufs=2, space="PSUM"))

        # one-time loads
        t_at = const.tile([128, T_loc * 4], F32, name="t_at")
        t_rl = const.tile([128, T_loc], F32, name="t_rl")
        t_io = const.tile([128, 128], BF16, name="t_io")
        t_w1 = const.tile([64, 128], BF16, name="t_w1")
        t_w2 = const.tile([128, 128], BF16, name="t_w2")
        t_wg = const.tile([128, 256], BF16, name="t_wg")
        t_b1 = const.tile([128, 1], F32, name="t_b1")
        t_b2 = const.tile([128, 1], F32, name="t_b2")
        t_bg = const.tile([1, 256], BF16, name="t_bg")
        t_ones = const.tile([1, 128], BF16, name="t_ones")
        nc.sync.dma_start(t_at[:], d_at[:])
        nc.sync.dma_start(t_rl[:], d_rl[:])
        nc.sync.dma_start(t_io[:], d_io[:])
        nc.sync.dma_start(t_w1[:], d_w1[:])
        nc.sync.dma_start(t_w2[:], d_w2[:])
        nc.sync.dma_start(t_wg[:], d_wg[:])
        nc.sync.dma_start(t_b1[:], d_b1[:])
        nc.sync.dma_start(t_b2[:], d_b2[:])
        nc.sync.dma_start(t_bg[:], d_bg[:])
        nc.vector.memset(t_ones[:], 1.0)

        loop_ctx = tc.For_i(0, repeat, 1) if repeat > 1 else None
        if loop_ctx is not None:
            ctx.enter_context(loop_ctx)
        for b in range(B_max):
            p_blk = ps_blk.tile([128, 512], F32, name=f"p_blk{b}", tag="p_blk")
            for st_in_b in range(BK // ST_TILES):
                st = b * (BK // ST_TILES) + st_in_b
                e0 = st * ST_TILES * TILE  # first edge of supertile

                # ---- loads
                t_sT = io_pool.tile([64, ST_E], BF16, name=f"sT{st}", tag="sT")
                nc.sync.dma_start(t_sT[:], d_sT[:, e0:e0 + ST_E])
                t_sA = io_pool.tile([128, ST_TILES * 64], BF16, name=f"sA{st}", tag="sA")
                nc.sync.dma_start(
                    t_sA[:], d_sA[:, st * ST_TILES * 64:(st + 1) * ST_TILES * 64])
                t_vA = io_pool.tile([128, ST_TILES * 192], BF16, name=f"vA{st}", tag="vA")
                nc.sync.dma_start(
                    t_vA[:], d_vA[:, st * ST_TILES * 192:(st + 1) * ST_TILES * 192])

                # ---- MLP (feature-on-partition, bf16)
                p_h1 = ps_mlp.tile([128, ST_E], F32, name=f"ph1_{st}", tag="p_mlp")
                for hh in range(ST_E // 512):
                    nc.tensor.matmul(p_h1[:, hh * 512:(hh + 1) * 512], t_w1[:],
                                     t_sT[:, hh * 512:(hh + 1) * 512],
                                     start=True, stop=True)
                t_h1 = mlp_pool.tile([128, ST_E], BF16, name=f"h1_{st}", tag="h1")
                nc.scalar.activation(t_h1[:], p_h1[:], AF.Silu, bias=t_b1[:, 0:1])
                p_h2 = ps_mlp.tile([128, ST_E], F32, name=f"ph2_{st}", tag="p_mlp")
                for hh in range(ST_E // 512):
                    nc.tensor.matmul(p_h2[:, hh * 512:(hh + 1) * 512], t_w2[:],
                                     t_h1[:, hh * 512:(hh + 1) * 512],
                                     start=True, stop=True)
                t_h2 = mlp_pool.tile([128, ST_E], BF16, name=f"h2_{st}", tag="h2")
                nc.scalar.activation(t_h2[:], p_h2[:], AF.Silu, bias=t_b2[:, 0:1])

                # gate pairs: two subtiles share one [128x512] PSUM bank so
                # the ACT copy runs once per pair
                use_bias = OPT.get("gate_bias", True)
                gpair = []
                for half in range(ST_TILES // 2):
                    p_g2 = ps_gate.tile([128, 512], F32, name=f"pg{st}_{half}",
                                        tag="p_g")
                    for q in range(2):
                        s = half * 2 + q
                        nc.tensor.matmul(
                            p_g2[:, q * 256:(q + 1) * 256],
                            t_h2[:, s * 128:(s + 1) * 128], t_wg[:],
                            start=True, stop=not use_bias)
                        if use_bias:
                            nc.tensor.matmul(
                                p_g2[:, q * 256:(q + 1) * 256], t_ones[:],
                                t_bg[:], start=False, stop=True)
                    t_g2 = gate_pool.tile([128, 512], BF16, name=f"g{st}_{half}",
                                          tag="g")
                    nc.scalar.activation(t_g2[:], p_g2[:], AF.Copy)
                    gpair.append(t_g2)

                for s in range(ST_TILES):
                    t = st * ST_TILES + s            # global tile index
                    tb = st_in_b * ST_TILES + s      # tile index within block
                    t_g = gpair[s // 2][:, (s % 2) * 256:(s % 2 + 1) * 256]

                    sA = t_sA[:, s * 64:(s + 1) * 64]
                    vA = t_vA[:, s * 192:(s + 1) * 192]
                    a_s = t_at[:, 4 * t + 0:4 * t + 1]
                    av = [t_at[:, 4 * t + 1 + c:4 * t + 2 + c] for c in range(3)]

                    t_msg = msg_pool.tile([128, 512], BF16, name=f"m{t}", tag="m")
                    # m0a = (s * as) * g0a
                    nc.vector.scalar_tensor_tensor(
                        t_msg[:, 0:64], sA, a_s, t_g[:, 0:64], ALU.mult, ALU.mult)
                    # m0b = (sum_c ev_c * av_c) * g0b   (CG factor folded in Wg)
                    t_u = tmp_pool.tile([128, 64], BF16, name=f"u{t}", tag="u")
                    t_v = tmp_pool.tile([128, 64], BF16, name=f"v{t}", tag="v")
                    t_w = tmp_pool.tile([128, 64], BF16, name=f"w{t}", tag="w")
                    nc.vector.tensor_scalar(t_u[:], vA[:, 0:64], av[0], None, ALU.mult)
                    nc.vector.scalar_tensor_tensor(
                        t_v[:], vA[:, 64:128], av[1], t_u[:], ALU.mult, ALU.add)
                    nc.vector.scalar_tensor_tensor(
                        t_w[:], vA[:, 128:192], av[2], t_v[:], ALU.mult, ALU.add)
                    nc.vector.tensor_tensor(
                        t_msg[:, 64:128], t_w[:], t_g[:, 64:128], ALU.mult)
                    # m1a planes: (s * av_c) * g1a
                    m1a_eng = nc.gpsimd if OPT.get("m1a_gpsimd") else nc.vector
                    for c in range(3):
                        m1a_eng.scalar_tensor_tensor(
                            t_msg[:, 128 + 64 * c:192 + 64 * c], sA, av[c],
                            t_g[:, 128:192], ALU.mult, ALU.mult)
                    # m1b = (ev * as) * g1b (repeated per plane)
                    m1b_eng = nc.gpsimd if OPT.get("m1b_gpsimd") else nc.vector
                    g1b_rep = t_g[:, 192:256].unsqueeze(1).broadcast_to((128, 3, 64))
                    m1b_eng.scalar_tensor_tensor(
                        t_msg[:, 320:512].rearrange("p (c v) -> p c v", c=3),
                        vA.rearrange("p (c v) -> p c v", c=3),
                        a_s, g1b_rep, ALU.mult, ALU.mult)

                    # ---- scatter: one-hot matmul accumulate into block PSUM
                    t_oh = tmp_pool.tile([128, 128], BF16, name=f"oh{t}", tag="oh")
                    nc.vector.tensor_scalar(
                        t_oh[:], t_io[:], t_rl[:, t:t + 1], None, ALU.is_equal)
                    nc.tensor.matmul(p_blk[:], t_oh[:], t_msg[:],
                                     start=(tb == 0), stop=(tb == BK - 1))

            # ---- flush block
            t_ob = out_pool.tile([128, 512], F32, name=f"ob{b}", tag="ob")
            nc.scalar.activation(t_ob[:], p_blk[:], AF.Copy)
            nc.sync.dma_start(d_out[b * 128:(b + 1) * 128, :], t_ob[:])

    nc.compile()
    return nc


_PROG_CACHE = {}


def _get_program(B_max, T_loc, gate_bias):
    key = (B_max, T_loc, gate_bias)
    if key not in _PROG_CACHE:
        OPT["gate_bias"] = gate_bias
        _PROG_CACHE[key] = _build_program(B_max, T_loc)
    return _PROG_CACHE[key]


def kernel(edge_s, edge_v, attr_s, attr_v, W1, b1, W2, b2, Wg, bg,
           receivers, n_nodes):
    edge_s = np.asarray(edge_s, np.float32)
    edge_v = np.asarray(edge_v, np.float32)
    attr_s = np.asarray(attr_s, np.float32)
    attr_v = np.asarray(attr_v, np.float32)
    W1 = np.asarray(W1, np.float32)
    b1 = np.asarray(b1, np.float32)
    W2 = np.asarray(W2, np.float32)
    b2 = np.asarray(b2, np.float32)
    Wg = np.asarray(Wg, np.float32)
    bg = np.asarray(bg, np.float32)
    receivers = np.asarray(receivers, np.int32)
    n_nodes = int(np.asarray(n_nodes))
    e_total = receivers.shape[0]

    # fold normalization + CG factor into the gate weights
    scale = np.full((256,), 1.0 / np.sqrt(AVG_NUM_NEIGHBORS), np.float32)
    scale[64:128] *= INV_SQRT3
    Wg_f = Wg * scale[None, :]
    bg_f = bg * scale

    # ---- sort by receiver, shard, block
    perm = np.argsort(receivers, kind="stable")
    recv_sorted = receivers[perm]
    cuts = [round(i * e_total / N_CORES) for i in range(N_CORES + 1)]
    core_blocks = [_build_blocks(recv_sorted, cuts[i], cuts[i + 1])
                   for i in range(N_CORES)]
    B_max = max(len(cb) for cb in core_blocks)
    T_loc = B_max * BK
    E_loc = T_loc * TILE

    # ---- per-core packed arrays
    in_maps = []
    meta = []  # per core: list of base nodes
    for ci in range(N_CORES):
        eidx = np.zeros((E_loc,), np.int64)      # gathered edge index (perm'd)
        valid = np.zeros((E_loc,), bool)
        rloc = np.zeros((E_loc,), np.float32)
        bases = []
        for bi, (base, i0, i1) in enumerate(core_blocks[ci]):
            n = i1 - i0
            sl = slice(bi * BLK_EDGES, bi * BLK_EDGES + n)
            eidx[sl] = perm[i0:i1]
            valid[sl] = True
            rloc[sl] = (recv_sorted[i0:i1] - base).astype(np.float32)
            bases.append(base)
        bases += [0] * (B_max - len(bases))
        meta.append(bases)

        es = edge_s[eidx]                       # [E_loc, 64]
        es[~valid] = 0.0
        ev = edge_v[eidx]                       # [E_loc, 64, 3]
        ev[~valid] = 0.0
        a_s = attr_s[eidx, 0]
        a_s[~valid] = 0.0
        a_v = attr_v[eidx]                      # [E_loc, 3]
        a_v[~valid] = 0.0

        ev_pm = np.ascontiguousarray(ev.transpose(0, 2, 1))   # [E_loc, 3, 64]
        attrs4 = np.concatenate([a_s[:, None], a_v], axis=1)  # [E_loc, 4]

        in_maps.append({
            "edge_sT": _to_bf16(np.ascontiguousarray(es.T)),
            "edge_sA": _to_bf16(
                es.reshape(T_loc, TILE, 64).transpose(1, 0, 2).reshape(128, -1)),
            "edge_vA": _to_bf16(
                ev_pm.reshape(T_loc, TILE, 192).transpose(1, 0, 2).reshape(128, -1)),
            "attrs": np.ascontiguousarray(
                attrs4.reshape(T_loc, TILE, 4).transpose(1, 0, 2).reshape(128, -1)),
            "rloc": np.ascontiguousarray(rloc.reshape(T_loc, TILE).T),
            "iota": _to_bf16(np.broadcast_to(
                np.arange(128, dtype=np.float32), (128, 128))),
            "W1": _to_bf16(W1),
            "W2": _to_bf16(W2),
            "Wg": _to_bf16(Wg_f),
            "b1": b1.reshape(128, 1).astype(np.float32),
            "b2": b2.reshape(128, 1).astype(np.float32),
            "bgr": _to_bf16(bg_f.reshape(1, 256)),
        })

    nc = _get_program(B_max, T_loc, gate_bias=bool(np.any(bg_f != 0)))
    res = bass_utils.run_bass_kernel_spmd(nc, in_maps, list(range(N_CORES)))

    # ---- host combine: add block slabs, reorder m1 columns
    full = np.zeros((n_nodes + 128, 512), np.float32)
    for ci in range(N_CORES):
        slab = res.results[ci]["out"]
        for bi, base in enumerate(meta[ci]):
            if bi < len(core_blocks[ci]):
                full[base:base + 128] += slab[bi * 128:(bi + 1) * 128]
    full = full[:n_nodes]

    colperm = np.arange(512)
    v = np.arange(64)
    for c in range(3):
        colperm[128 + 3 * v + c] = 128 + 64 * c + v    # m1a
        colperm[320 + 3 * v + c] = 320 + 64 * c + v    # m1b
    return np.ascontiguousarray(full[:, colperm])


# revision 20
# speedup vs baseline: 9812.9063x; 1.1447x over previous
"""Trainium2 Bass kernel for nn_MessagePassingConvolution (GNN message passing).

Strategy (8 NeuronCores, SPMD):
  * Host: sort edges by receiver (CSR-style), shard the sorted edge stream
    evenly across 8 cores, group each core's edges into node-blocks (<=128
    distinct consecutive node ids per block, padded to a fixed tile count so
    one program serves all cores).
  * Device per core: edge-tile pipeline -
      - MLP gate: feature-on-partition bf16 matmuls (W1/W2), per-edge gate
        via h2-subtile-as-stationary matmul (+ ones x bg bias matmul),
      - equivariant tensor product + gating on VectorE (fused
        scalar_tensor_tensor ops, per-partition attr scalars),
      - scatter-add: one-hot (is_equal vs iota) matmul accumulating into a
        PSUM bank per node-block; flushed PSUM->SBUF->HBM per block.
  * Host: sum per-block 128-row slabs into the [N,512] output (few adds),
    reorder m1 columns to the reference (f-major, c-minor) layout.
  The 1/sqrt(avg_neighbors) normalization and the 1o x 1o -> 0e CG factor are
  folded into Wg/bg, so no extra device work.
"""

import sys

sys.path.insert(0, "/opt/trn_rl_repo")

import numpy as np
from contextlib import ExitStack

from concourse import bacc, tile, bass_utils, mybir

F32 = mybir.dt.float32
BF16 = mybir.dt.bfloat16
AF = mybir.ActivationFunctionType
ALU = mybir.AluOpType

E = 160000
N_NODES = 10000
INV_SQRT3 = 0.5773502691896258
AVG_NUM_NEIGHBORS = 16.0
N_CORES = 8
TILE = 128           # edges per tile (= scatter matmul K)
BK = 8               # tiles per node-block (must be divisible by ST_TILES)
ST_TILES = 8         # tiles per supertile (MLP granularity, 1024 edges)
BLK_EDGES = BK * TILE
ST_E = ST_TILES * TILE

_BF = np.dtype(mybir.dt.np(BF16))


def _to_bf16(x):
    return x.astype(_BF)


# ---------------------------------------------------------------- host prep


def _build_blocks(recv_sorted, lo, hi):
    """Greedy blocking of the sorted edge range [lo, hi): each block covers
    < 128 distinct node ids and at most BLK_EDGES edges. Returns list of
    (base_node, edge_start, edge_end)."""
    blocks = []
    i = lo
    while i < hi:
        base = int(recv_sorted[i])
        limit = np.searchsorted(recv_sorted[lo:hi], base + 128, side="left") + lo
        end = min(i + BLK_EDGES, limit, hi)
        blocks.append((base, i, int(end)))
        i = int(end)
    return blocks


OPT = {}


def _build_program(B_max, T_loc, repeat=1):
    """Build the SPMD Bass program: B_max blocks x BK tiles per core.

    repeat > 1 wraps the whole compute in an on-device loop (timing only)."""
    nc = bacc.Bacc("TRN2", target_bir_lowering=False, debug=False,
                   num_devices=N_CORES)
    E_loc = T_loc * TILE

    d_sT = nc.dram_tensor("edge_sT", [64, E_loc], BF16, kind="ExternalInput").ap()
    d_sA = nc.dram_tensor("edge_sA", [128, T_loc * 64], BF16, kind="ExternalInput").ap()
    d_vA = nc.dram_tensor("edge_vA", [128, T_loc * 192], BF16, kind="ExternalInput").ap()
    d_at = nc.dram_tensor("attrs", [128, T_loc * 4], F32, kind="ExternalInput").ap()
    d_rl = nc.dram_tensor("rloc", [128, T_loc], F32, kind="ExternalInput").ap()
    d_io = nc.dram_tensor("iota", [128, 128], BF16, kind="ExternalInput").ap()
    d_w1 = nc.dram_tensor("W1", [64, 128], BF16, kind="ExternalInput").ap()
    d_w2 = nc.dram_tensor("W2", [128, 128], BF16, kind="ExternalInput").ap()
    d_wg = nc.dram_tensor("Wg", [128, 256], BF16, kind="ExternalInput").ap()
    d_b1 = nc.dram_tensor("b1", [128, 1], F32, kind="ExternalInput").ap()
    d_b2 = nc.dram_tensor("b2", [128, 1], F32, kind="ExternalInput").ap()
    d_bg = nc.dram_tensor("bgr", [1, 256], BF16, kind="ExternalInput").ap()
    d_out = nc.dram_tensor("out", [B_max * 128, 512], F32, kind="ExternalOutput").ap()

    with tile.TileContext(nc) as tc, ExitStack() as ctx:
        const = ctx.enter_context(tc.tile_pool(name="const", bufs=1))
        io_pool = ctx.enter_context(tc.tile_pool(name="io", bufs=3))
        mlp_pool = ctx.enter_context(tc.tile_pool(name="mlp", bufs=2))
        gate_pool = ctx.enter_context(tc.tile_pool(name="gate", bufs=4))
        msg_pool = ctx.enter_context(tc.tile_pool(name="msg", bufs=4))
        tmp_pool = ctx.enter_context(tc.tile_pool(name="tmp", bufs=4))
        out_pool = ctx.enter_context(tc.tile_pool(name="outp", bufs=2))
        ps_mlp = ctx.enter_context(tc.tile_pool(name="ps_mlp", bufs=2, space="PSUM"))
        ps_gate = ctx.enter_context(tc.tile_pool(name="ps_gate", bufs=2, space="PSUM"))
        ps_blk = ctx.enter_context(tc.tile_pool(name="ps_blk", bufs=2, space="PSUM"))

        # one-time loads
        t_at = const.tile([128, T_loc * 4], F32, name="t_at")
        t_rl = const.tile([128, T_loc], F32, name="t_rl")
        t_io = const.tile([128, 128], BF16, name="t_io")
        t_w1 = const.tile([64, 128], BF16, name="t_w1")
        t_w2 = const.tile([128, 128], BF16, name="t_w2")
        t_wg = const.tile([128, 256], BF16, name="t_wg")
        t_b1 = const.tile([128, 1], F32, name="t_b1")
        t_b2 = const.tile([128, 1], F32, name="t_b2")
        t_bg = const.tile([1, 256], BF16, name="t_bg")
        t_ones = const.tile([1, 128], BF16, name="t_ones")
        nc.sync.dma_start(t_at[:], d_at[:])
        nc.sync.dma_start(t_rl[:], d_rl[:])
        nc.sync.dma_start(t_io[:], d_io[:])
        nc.sync.dma_start(t_w1[:], d_w1[:])
        nc.sync.dma_start(t_w2[:], d_w2[:])
        nc.sync.dma_start(t_wg[:], d_wg[:])
        nc.sync.dma_start(t_b1[:], d_b1[:])
        nc.sync.dma_start(t_b2[:], d_b2[:])
        nc.sync.dma_start(t_bg[:], d_bg[:])
        nc.vector.memset(t_ones[:], 1.0)

        loop_ctx = tc.For_i(0, repeat, 1) if repeat > 1 else None
        if loop_ctx is not None:
            ctx.enter_context(loop_ctx)
        for b in range(B_max):
            p_blk = ps_blk.tile([128, 512], F32, name=f"p_blk{b}", tag="p_blk")
            for st_in_b in range(BK // ST_TILES):
                st = b * (BK // ST_TILES) + st_in_b
                e0 = st * ST_TILES * TILE  # first edge of supertile

                # ---- loads
                t_sT = io_pool.tile([64, ST_E], BF16, name=f"sT{st}", tag="sT")
                nc.sync.dma_start(t_sT[:], d_sT[:, e0:e0 + ST_E])
                t_sA = io_pool.tile([128, ST_TILES * 64], BF16, name=f"sA{st}", tag="sA")
                nc.sync.dma_start(
                    t_sA[:], d_sA[:, st * ST_TILES * 64:(st + 1) * ST_TILES * 64])
                t_vA = io_pool.tile([128, ST_TILES * 192], BF16, name=f"vA{st}", tag="vA")
                nc.sync.dma_start(
                    t_vA[:], d_vA[:, st * ST_TILES * 192:(st + 1) * ST_TILES * 192])

                # ---- MLP (feature-on-partition, bf16)
                p_h1 = ps_mlp.tile([128, ST_E], F32, name=f"ph1_{st}", tag="p_mlp")
                for hh in range(ST_E // 512):
                    nc.tensor.matmul(p_h1[:, hh * 512:(hh + 1) * 512], t_w1[:],
                                     t_sT[:, hh * 512:(hh + 1) * 512],
                                     start=True, stop=True)
                t_h1 = mlp_pool.tile([128, ST_E], BF16, name=f"h1_{st}", tag="h1")
                nc.scalar.activation(t_h1[:], p_h1[:], AF.Silu, bias=t_b1[:, 0:1])
                p_h2 = ps_mlp.tile([128, ST_E], F32, name=f"ph2_{st}", tag="p_mlp")
                for hh in range(ST_E // 512):
                    nc.tensor.matmul(p_h2[:, hh * 512:(hh + 1) * 512], t_w2[:],
                                     t_h1[:, hh * 512:(hh + 1) * 512],
                                     start=True, stop=True)
                t_h2 = mlp_pool.tile([128, ST_E], BF16, name=f"h2_{st}", tag="h2")
                nc.scalar.activation(t_h2[:], p_h2[:], AF.Silu, bias=t_b2[:, 0:1])

                # gate pairs: two subtiles share one [128x512] PSUM bank so
                # the ACT copy runs once per pair
                use_bias = OPT.get("gate_bias", True)
                gpair = []
                for half in range(ST_TILES // 2):
                    p_g2 = ps_gate.tile([128, 512], F32, name=f"pg{st}_{half}",
                                        tag="p_g")
                    for q in range(2):
                        s = half * 2 + q
                        nc.tensor.matmul(
                            p_g2[:, q * 256:(q + 1) * 256],
                            t_h2[:, s * 128:(s + 1) * 128], t_wg[:],
                            start=True, stop=not use_bias)
                        if use_bias:
                            nc.tensor.matmul(
                                p_g2[:, q * 256:(q + 1) * 256], t_ones[:],
                                t_bg[:], start=False, stop=True)
                    t_g2 = gate_pool.tile([128, 512], BF16, name=f"g{st}_{half}",
                                          tag="g")
                    nc.scalar.activation(t_g2[:], p_g2[:], AF.Copy)
                    gpair.append(t_g2)

                for s in range(ST_TILES):
                    t = st * ST_TILES + s            # global tile index
                    tb = st_in_b * ST_TILES + s      # tile index within block
                    t_g = gpair[s // 2][:, (s % 2) * 256:(s % 2 + 1) * 256]

                    sA = t_sA[:, s * 64:(s + 1) * 64]
                    vA = t_vA[:, s * 192:(s + 1) * 192]
                    a_s = t_at[:, 4 * t + 0:4 * t + 1]
                    av = [t_at[:, 4 * t + 1 + c:4 * t + 2 + c] for c in range(3)]

                    t_msg = msg_pool.tile([128, 512], BF16, name=f"m{t}", tag="m")
                    # m0a = (s * as) * g0a
                    nc.vector.scalar_tensor_tensor(
                        t_msg[:, 0:64], sA, a_s, t_g[:, 0:64], ALU.mult, ALU.mult)
                    # m0b = (sum_c ev_c * av_c) * g0b   (CG factor folded in Wg)
                    t_u = tmp_pool.tile([128, 64], BF16, name=f"u{t}", tag="u")
                    t_v = tmp_pool.tile([128, 64], BF16, name=f"v{t}", tag="v")
                    t_w = tmp_pool.tile([128, 64], BF16, name=f"w{t}", tag="w")
                    ts_eng = nc.gpsimd if OPT.get("gp_ts", True) else nc.vector
                    ts_eng.tensor_scalar(t_u[:], vA[:, 0:64], av[0], None, ALU.mult)
                    nc.vector.scalar_tensor_tensor(
                        t_v[:], vA[:, 64:128], av[1], t_u[:], ALU.mult, ALU.add)
                    nc.vector.scalar_tensor_tensor(
                        t_w[:], vA[:, 128:192], av[2], t_v[:], ALU.mult, ALU.add)
                    tt_eng = nc.gpsimd if OPT.get("gp_tt", True) else nc.vector
                    tt_eng.tensor_tensor(
                        t_msg[:, 64:128], t_w[:], t_g[:, 64:128], ALU.mult)
                    # m1a planes: (s * av_c) * g1a
                    if OPT.get("m1a_split_gp"):
                        t_m1a = tmp_pool.tile([128, 192], BF16, name=f"q{t}", tag="q")
                        for c in range(3):
                            nc.gpsimd.tensor_scalar(
                                t_m1a[:, 64 * c:64 * c + 64], sA, av[c], None, ALU.mult)
                            nc.gpsimd.tensor_tensor(
                                t_msg[:, 128 + 64 * c:192 + 64 * c],
                                t_m1a[:, 64 * c:64 * c + 64],
                                t_g[:, 128:192], ALU.mult)
                    else:
                        for c in range(3):
                            nc.vector.scalar_tensor_tensor(
                                t_msg[:, 128 + 64 * c:192 + 64 * c], sA, av[c],
                                t_g[:, 128:192], ALU.mult, ALU.mult)
                    # m1b = (ev * as) * g1b (repeated per plane)
                    m1b_eng = nc.gpsimd if OPT.get("m1b_gpsimd") else nc.vector
                    g1b_rep = t_g[:, 192:256].unsqueeze(1).broadcast_to((128, 3, 64))
                    m1b_eng.scalar_tensor_tensor(
                        t_msg[:, 320:512].rearrange("p (c v) -> p c v", c=3),
                        vA.rearrange("p (c v) -> p c v", c=3),
                        a_s, g1b_rep, ALU.mult, ALU.mult)

                    # ---- scatter: one-hot matmul accumulate into block PSUM
                    t_oh = tmp_pool.tile([128, 128], BF16, name=f"oh{t}", tag="oh")
                    oh_eng = nc.gpsimd if OPT.get("gp_oh") else nc.vector
                    oh_eng.tensor_scalar(
                        t_oh[:], t_io[:], t_rl[:, t:t + 1], None, ALU.is_equal)
                    nc.tensor.matmul(p_blk[:], t_oh[:], t_msg[:],
                                     start=(tb == 0), stop=(tb == BK - 1))

            # ---- flush block
            t_ob = out_pool.tile([128, 512], F32, name=f"ob{b}", tag="ob")
            nc.scalar.activation(t_ob[:], p_blk[:], AF.Copy)
            nc.sync.dma_start(d_out[b * 128:(b + 1) * 128, :], t_ob[:])

    nc.compile()
    return nc


_PROG_CACHE = {}


def _get_program(B_max, T_loc, gate_bias):
    key = (B_max, T_loc, gate_bias)
    if key not in _PROG_CACHE:
        OPT["gate_bias"] = gate_bias
        _PROG_CACHE[key] = _build_program(B_max, T_loc)
    return _PROG_CACHE[key]


def kernel(edge_s, edge_v, attr_s, attr_v, W1, b1, W2, b2, Wg, bg,
           receivers, n_nodes):
    edge_s = np.asarray(edge_s, np.float32)
    edge_v = np.asarray(edge_v, np.float32)
    attr_s = np.asarray(attr_s, np.float32)
    attr_v = np.asarray(attr_v, np.float32)
    W1 = np.asarray(W1, np.float32)
    b1 = np.asarray(b1, np.float32)
    W2 = np.asarray(W2, np.float32)
    b2 = np.asarray(b2, np.float32)
    Wg = np.asarray(Wg, np.float32)
    bg = np.asarray(bg, np.float32)
    receivers = np.asarray(receivers, np.int32)
    n_nodes = int(np.asarray(n_nodes))
    e_total = receivers.shape[0]

    # fold normalization + CG factor into the gate weights
    scale = np.full((256,), 1.0 / np.sqrt(AVG_NUM_NEIGHBORS), np.float32)
    scale[64:128] *= INV_SQRT3
    Wg_f = Wg * scale[None, :]
    bg_f = bg * scale

    # ---- sort by receiver, shard, block
    perm = np.argsort(receivers, kind="stable")
    recv_sorted = receivers[perm]
    cuts = [round(i * e_total / N_CORES) for i in range(N_CORES + 1)]
    core_blocks = [_build_blocks(recv_sorted, cuts[i], cuts[i + 1])
                   for i in range(N_CORES)]
    B_max = max(len(cb) for cb in core_blocks)
    T_loc = B_max * BK
    E_loc = T_loc * TILE

    # ---- per-core packed arrays
    in_maps = []
    meta = []  # per core: list of base nodes
    for ci in range(N_CORES):
        eidx = np.zeros((E_loc,), np.int64)      # gathered edge index (perm'd)
        valid = np.zeros((E_loc,), bool)
        rloc = np.zeros((E_loc,), np.float32)
        bases = []
        for bi, (base, i0, i1) in enumerate(core_blocks[ci]):
            n = i1 - i0
            sl = slice(bi * BLK_EDGES, bi * BLK_EDGES + n)
            eidx[sl] = perm[i0:i1]
            valid[sl] = True
            rloc[sl] = (recv_sorted[i0:i1] - base).astype(np.float32)
            bases.append(base)
        bases += [0] * (B_max - len(bases))
        meta.append(bases)

        es = edge_s[eidx]                       # [E_loc, 64]
        es[~valid] = 0.0
        ev = edge_v[eidx]                       # [E_loc, 64, 3]
        ev[~valid] = 0.0
        a_s = attr_s[eidx, 0]
        a_s[~valid] = 0.0
        a_v = attr_v[eidx]                      # [E_loc, 3]
        a_v[~valid] = 0.0

        ev_pm = np.ascontiguousarray(ev.transpose(0, 2, 1))   # [E_loc, 3, 64]
        attrs4 = np.concatenate([a_s[:, None], a_v], axis=1)  # [E_loc, 4]

        in_maps.append({
            "edge_sT": _to_bf16(np.ascontiguousarray(es.T)),
            "edge_sA": _to_bf16(
                es.reshape(T_loc, TILE, 64).transpose(1, 0, 2).reshape(128, -1)),
            "edge_vA": _to_bf16(
                ev_pm.reshape(T_loc, TILE, 192).transpose(1, 0, 2).reshape(128, -1)),
            "attrs": np.ascontiguousarray(
                attrs4.reshape(T_loc, TILE, 4).transpose(1, 0, 2).reshape(128, -1)),
            "rloc": np.ascontiguousarray(rloc.reshape(T_loc, TILE).T),
            "iota": _to_bf16(np.broadcast_to(
                np.arange(128, dtype=np.float32), (128, 128))),
            "W1": _to_bf16(W1),
            "W2": _to_bf16(W2),
            "Wg": _to_bf16(Wg_f),
            "b1": b1.reshape(128, 1).astype(np.float32),
            "b2": b2.reshape(128, 1).astype(np.float32),
            "bgr": _to_bf16(bg_f.reshape(1, 256)),
        })

    nc = _get_program(B_max, T_loc, gate_bias=bool(np.any(bg_f != 0)))
    res = bass_utils.run_bass_kernel_spmd(nc, in_maps, list(range(N_CORES)))

    # ---- host combine: add block slabs, reorder m1 columns
    full = np.zeros((n_nodes + 128, 512), np.float32)
    for ci in range(N_CORES):
        slab = res.results[ci]["out"]
        for bi, base in enumerate(meta[ci]):
            if bi < len(core_blocks[ci]):
                full[base:base + 128] += slab[bi * 128:(bi + 1) * 128]
    full = full[:n_nodes]

    colperm = np.arange(512)
    v = np.arange(64)
    for c in range(3):
        colperm[128 + 3 * v + c] = 128 + 64 * c + v    # m1a
        colperm[320 + 3 * v + c] = 320 + 64 * c + v    # m1b
    return np.ascontiguousarray(full[:, colperm])
